# revision 16
# baseline (speedup 1.0000x reference)
"""MatchFilter (graph-pair cross-attention + gated segment sum) on 8 trn2 cores.

Math per graph pair b (reference):
    S = L_b @ R_b^T                      [nl, nr]
    P_row = softmax(S, axis=1);  P_col = softmax(S, axis=0)
    wl_i = sigmoid(<L_i, (P_row @ R)_i>) = sigmoid( (sum_j E_ij S_ij) / (sum_j E_ij) )
    wr_j analogously from S^T.
    out_l[b] = sum_i wl_i L_i ;  out_r[b] = sum_j wr_j R_j

Device algorithm (identities that keep every reduction cheap):
  * <L_i, right_atten_i> = (sum_j E_ij S_ij) / (sum_j E_ij): only row/col
    stats of S are needed; the attention matmul is never materialized.
  * Everything is computed from the SINGLE orientation e = exp(S - c):
      s_R = colsum(e), t_R = colsum(e*S)  -> nearly-free N=1 PE matmuls
            against a ones vector (contraction along partitions);
      t_L = rowsum(e*S) -> free accum_out of the per-pair DVE product;
      s_L = rowsum(e)   -> per-pair copy-with-accum on the otherwise idle
            GPSIMD engine (reads e from SBUF; GPSIMD cannot touch PSUM).
  * sigmoid(z) = 0.5 + 0.5*tanh(z/2); the affine is folded into the output
    matmuls: out[b] = 0.5*(sum_i L_i + sum_i tanh_i L_i) via two accumulated
    N=1 matmuls per (pair, side, d-chunk), and the 0.5 into the final
    PSUM->SBUF copy.
  * Score matmuls run in fp8e4 DoubleRow: one matmul per pair does the full
    256-deep contraction.  Score quantization (~1 unit of S) is harmless:
    z sits 30+ sigma into sigmoid saturation for S ~ N(0,256), while the
    final weighted sum uses exact fp16 embeddings.

Sharding: 64 pairs -> 8 cores x 8 pairs, fully local (data parallel over
pairs).  Host pre-swizzles per-core inputs: d-major fp8 for scores, node-major
fp16 for the output matmuls.  Input DMAs are spread over the DMA-capable
queues (SP, Pool), score halves first so compute starts early.
"""

import os
import numpy as np
from contextlib import ExitStack

import concourse.bass as bass
import concourse.bacc as bacc
import concourse.tile as tile
from concourse import mybir
from concourse.bass_utils import run_bass_kernel_spmd

N_CORES = 8
B = 64            # graph pairs
D = 256           # embedding dim
DC = D // 128     # d-chunks for the 128-deep contraction
PAIRS_PER_CORE = B // N_CORES
# exp(S - EXP_SHIFT): keeps e^S and its row-sums inside f32 range for
# S ~ N(0, 256) (|S| <~ 95).  Max-subtraction is unnecessary because the
# z = t/s ratio is shift-invariant.
EXP_SHIFT = 32.0

LAST_RESULT = None  # BassKernelResults of the most recent run (for test.py)
LAST_TIMING = {}
LAST_IN_MAPS = []

_NC_CACHE = {}


def _build_bass(pairs: int, p: int):
    """Per-core program: `pairs` graph pairs, each padded to `p` nodes/side."""
    f16, f32 = mybir.dt.float16, mybir.dt.float32
    f8 = mybir.dt.float8e4
    AF = mybir.ActivationFunctionType
    ALU = mybir.AluOpType
    nl = p // 128  # row-chunks per pair side
    assert nl == 1, "kernel specialized for 128-node pairs"
    half = pairs // 2

    nc = bacc.Bacc("TRN2", target_bir_lowering=False, debug=False,
                   num_devices=N_CORES)
    lt8 = nc.dram_tensor("lt8", [128, DC, pairs, p], f8, kind="ExternalInput").ap()
    rt8 = nc.dram_tensor("rt8", [128, DC, pairs, p], f8, kind="ExternalInput").ap()
    l_nat = nc.dram_tensor("l_nat", [128, pairs, D], f16, kind="ExternalInput").ap()
    r_nat = nc.dram_tensor("r_nat", [128, pairs, D], f16, kind="ExternalInput").ap()
    out_t = nc.dram_tensor("out_t", [128, DC, 2, pairs], f32,
                           kind="ExternalOutput").ap()

    with tile.TileContext(nc) as tc, ExitStack() as ctx:
        sb = ctx.enter_context(tc.tile_pool(name="sb", bufs=1))
        work = ctx.enter_context(tc.tile_pool(name="work", bufs=1))
        stat = ctx.enter_context(tc.tile_pool(name="stat", bufs=1))
        psum = ctx.enter_context(tc.tile_pool(name="psum", bufs=1, space="PSUM"))
        spsum = ctx.enter_context(tc.tile_pool(name="spsum", bufs=1, space="PSUM"))

        lt_sb = sb.tile([128, DC, pairs, p], f8, tag="lt_sb")
        rt_sb = sb.tile([128, DC, pairs, p], f8, tag="rt_sb")
        ln_sb = sb.tile([128, pairs, D], f16, tag="ln_sb")
        rn_sb = sb.tile([128, pairs, D], f16, tag="rn_sb")

        neg_shift = stat.tile([128, 1], f32, tag="neg_shift")
        ones = stat.tile([128, 1], f32, tag="ones")
        ones16 = stat.tile([128, 1], f16, tag="ones16")
        nc.vector.memset(neg_shift, -EXP_SHIFT)
        nc.vector.memset(ones, 1.0)
        nc.vector.memset(ones16, 1.0)

        # input DMAs: score halves first on both queues so the first-half
        # scores are visible at the earliest possible time; nat tensors follow
        h0, h1 = slice(0, half), slice(half, pairs)
        nc.sync.dma_start(out=lt_sb[:, :, h0, :], in_=lt8[:, :, h0, :])
        nc.sync.dma_start(out=lt_sb[:, :, h1, :], in_=lt8[:, :, h1, :])
        nc.gpsimd.dma_start(out=rt_sb[:, :, h0, :], in_=rt8[:, :, h0, :])
        nc.gpsimd.dma_start(out=rt_sb[:, :, h1, :], in_=rt8[:, :, h1, :])
        nc.sync.dma_start(out=ln_sb, in_=l_nat)
        nc.gpsimd.dma_start(out=rn_sb, in_=r_nat)

        # dummy activation: pulls the exp_and_others ACT table load (Exp+Tanh)
        # off the critical path (overlaps the input DMA wait)
        warm = stat.tile([128, 1], f32, tag="warm")
        nc.scalar.activation(out=warm, in_=neg_shift, func=AF.Exp,
                             bias=neg_shift, scale=1.0)

        # right-side stats + s_L in one PSUM tile (PE colsum outputs);
        # t_L lands in SBUF via DVE accum_out
        stats = spsum.tile([128, 3, pairs], f32, tag="stats")
        stat_sl, stat_sr, stat_tr = (stats[:, 0, :], stats[:, 1, :],
                                     stats[:, 2, :])
        outT = spsum.tile([128, DC, 2, pairs], f32, tag="outT")
        stat_tl = stat.tile([128, pairs], f32, tag="stat_tl")
        ht = stat.tile([128, 2, pairs], f16, tag="ht")

        DR = mybir.MatmulPerfMode.DoubleRow
        ss_all = psum.tile([128, pairs, p], f32, tag="ss_all")
        st_all = psum.tile([128, pairs, p], f32, tag="st_all")
        et_all = work.tile([128, pairs, p], f32, tag="et_all")

        for b in range(pairs):
            nc.tensor.matmul(ss_all[:, b, :], lhsT=lt_sb[:, :, b, :],
                             rhs=rt_sb[:, :, b, :], perf_mode=DR,
                             start=True, stop=True)
        # e = exp(S - c) in [2, 2, 4]-pair chunks: the small early chunks
        # (entirely inside the first DMA half) start the DVE product chain as
        # early as possible; emission interleaves exp -> products so tile-level
        # writer deps never stall the chain on a later chunk
        chunks = [(0, 2), (2, 4), (4, 8)] if pairs == 8 else [(0, pairs)]
        for ci, (c0, c1) in enumerate(chunks):
            # per-chunk tiles: a shared tile would add tile-granular WAR
            # stalls (a later chunk's write waiting on earlier chunks' readers)
            e_c = work.tile([128, c1 - c0, p], f32, tag=f"e_c{ci}")
            sc_c = work.tile([128, c1 - c0, p], f32, tag=f"sc_c{ci}")
            nc.scalar.activation(out=e_c, in_=ss_all[:, c0:c1, :],
                                 func=AF.Exp, bias=neg_shift, scale=1.0)
            for b in range(c0, c1):
                i = b - c0
                # e*S product; free accum_out = t_L rowsums
                nc.vector.scalar_tensor_tensor(
                    out=sc_c[:, i, :], in0=e_c[:, i, :], scalar=1.0,
                    in1=ss_all[:, b, :], op0=ALU.mult, op1=ALU.mult,
                    accum_out=stat_tl[:, b:b + 1])
                # s_R / t_R colsums: nearly-free N=1 matmuls
                nc.tensor.matmul(stat_sr[:, b:b + 1], lhsT=e_c[:, i, :],
                                 rhs=ones, start=True, stop=True)
                nc.tensor.matmul(stat_tr[:, b:b + 1], lhsT=sc_c[:, i, :],
                                 rhs=ones, start=True, stop=True)

        # S^T orientation, only for s_L = colsum(exp(S^T)): one batched
        # matmul group + one batched exp + free colsums
        for b in range(pairs):
            nc.tensor.matmul(st_all[:, b, :], lhsT=rt_sb[:, :, b, :],
                             rhs=lt_sb[:, :, b, :], perf_mode=DR,
                             start=True, stop=True)
        nc.scalar.activation(out=et_all, in_=st_all, func=AF.Exp,
                             bias=neg_shift, scale=1.0)
        for b in range(pairs):
            nc.tensor.matmul(stat_sl[:, b:b + 1], lhsT=et_all[:, b, :],
                             rhs=ones, start=True, stop=True)

        # gates: ht = tanh((t/s)/2); sigmoid affine folded into the output
        # matmuls + final scaled copy
        z = stat.tile([128, 2, pairs], f32, tag="z")
        rcp = stat.tile([128, 2, pairs], f32, tag="rcp")
        nc.vector.reciprocal(out=rcp[:, 1, :], in_=stat_sr)
        nc.vector.tensor_mul(z[:, 1, :], stat_tr, rcp[:, 1, :])
        nc.vector.reciprocal(out=rcp[:, 0, :], in_=stat_sl)
        nc.vector.tensor_mul(z[:, 0, :], stat_tl, rcp[:, 0, :])
        nc.scalar.activation(out=ht, in_=z, func=AF.Tanh, scale=0.5)

        # gated segment sums: per (pair, side, d-chunk) an accumulated pair of
        # N=1 matmuls: psum = sum_i L_i + sum_i tanh_i * L_i  (= 2*out)
        for b in range(pairs):
            for c in range(DC):
                for side, nat in ((0, ln_sb), (1, rn_sb)):
                    col = outT[:, c, side, b:b + 1]
                    lhsT = nat[:, b, c * 128:(c + 1) * 128]
                    nc.tensor.matmul(col, lhsT=lhsT, rhs=ones16,
                                     start=True, stop=False)
                    nc.tensor.matmul(col, lhsT=lhsT,
                                     rhs=ht[:, side, b:b + 1],
                                     start=False, stop=True)

        out_sb = stat.tile([128, DC, 2, pairs], f32, tag="out_sb")
        nc.vector.tensor_scalar(out=out_sb, in0=outT, scalar1=0.5,
                                scalar2=None, op0=ALU.mult)
        nc.sync.dma_start(out=out_t, in_=out_sb)

    nc.compile()
    return nc


def _bench_exec(nc, in_maps, reps):
    """Min wall time of the cached jitted 8-core NEFF dispatch (inputs
    pre-sharded on device; excludes jax tracing and input upload)."""
    import time as _time
    import jax
    import jax.numpy as jnp
    from jax.sharding import Mesh, PartitionSpec, NamedSharding
    from jax.experimental.shard_map import shard_map
    from concourse import bass2jax
    from concourse.bass2jax import _bass_exec_p

    n_cores = len(in_maps)
    part_name = nc.partition_id_tensor.name if nc.partition_id_tensor else None
    in_names, out_names, out_avals = [], [], []
    for alloc in nc.m.functions[0].allocations:
        if not isinstance(alloc, mybir.MemoryLocationSet):
            continue
        name = alloc.memorylocations[0].name
        if alloc.kind == "ExternalInput":
            if name != part_name:
                in_names.append(name)
        elif alloc.kind == "ExternalOutput":
            out_names.append(name)
            out_avals.append(jax.core.ShapedArray(
                tuple(alloc.tensor_shape), mybir.dt.np(alloc.dtype)))
    n_params = len(in_names)
    all_in_names = in_names + out_names
    if part_name is not None:
        all_in_names = all_in_names + [part_name]

    def _body(*args):
        operands = list(args)
        if part_name is not None:
            operands.append(bass2jax.partition_id_tensor())
        return tuple(_bass_exec_p.bind(
            *operands, out_avals=tuple(out_avals), in_names=tuple(all_in_names),
            out_names=tuple(out_names), lowering_input_output_aliases=(),
            sim_require_finite=True, sim_require_nnan=True, nc=nc))

    devices = jax.devices()[:n_cores]
    mesh = Mesh(np.asarray(devices), ("core",))
    spec = PartitionSpec("core")
    fn = jax.jit(shard_map(_body, mesh=mesh,
                           in_specs=(spec,) * (n_params + len(out_names)),
                           out_specs=(spec,) * len(out_names)),
                 keep_unused=True)
    sharding = NamedSharding(mesh, spec)
    dev_ins = [jax.device_put(
        np.concatenate([np.asarray(m[name]) for m in in_maps], axis=0), sharding)
        for name in in_names]
    dev_zeros = [jax.device_put(
        np.zeros((n_cores * a.shape[0], *a.shape[1:]), a.dtype), sharding)
        for a in out_avals]
    fn(*dev_ins, *dev_zeros)[0].block_until_ready()  # warm compile
    best = float("inf")
    for _ in range(reps):
        t0 = _time.perf_counter()
        outs = fn(*dev_ins, *dev_zeros)
        for o in outs:
            o.block_until_ready()
        best = min(best, _time.perf_counter() - t0)
    return best


def _noop_baseline(reps):
    """Min wall time of a near-empty program through the same run path —
    estimates the host/axon dispatch overhead included in kernel_wall_s."""
    import time as _time
    if "noop" not in _NC_CACHE:
        nc = bacc.Bacc("TRN2", target_bir_lowering=False, debug=False,
                       num_devices=N_CORES)
        x = nc.dram_tensor("x", [128, 16], mybir.dt.float32,
                           kind="ExternalInput").ap()
        y = nc.dram_tensor("y", [128, 16], mybir.dt.float32,
                           kind="ExternalOutput").ap()
        with tile.TileContext(nc) as tc, ExitStack() as ctx:
            pool = ctx.enter_context(tc.tile_pool(name="p", bufs=1))
            t = pool.tile([128, 16], mybir.dt.float32)
            nc.sync.dma_start(out=t, in_=x)
            nc.sync.dma_start(out=y, in_=t)
        nc.compile()
        _NC_CACHE["noop"] = nc
    nc = _NC_CACHE["noop"]
    ins = [{"x": np.zeros((128, 16), np.float32)} for _ in range(N_CORES)]
    return _bench_exec(nc, ins, reps)


def sim_time_ns(in_map, pairs, p):
    """CoreSim cost-model time for one core's program (ns)."""
    from concourse import bass_interp
    key = (pairs, p)
    if key not in _NC_CACHE:
        _NC_CACHE[key] = _build_bass(*key)
    sim = bass_interp.CoreSim(_NC_CACHE[key])
    for name, arr in in_map.items():
        sim.tensor(name)[:] = arr
    sim.simulate()
    return int(sim.time)


def _pack_side(emb, seg_id, p):
    """[N, D] ragged -> [B, p, D] zero-padded f32 (no-copy reshape if uniform)."""
    counts = np.bincount(seg_id, minlength=B)
    if (counts == p).all():
        return emb.reshape(B, p, D)
    out = np.zeros((B, p, D), emb.dtype)
    offs = np.concatenate([[0], np.cumsum(counts)])
    for g in range(B):
        out[g, :counts[g]] = emb[offs[g]:offs[g + 1]]
    return out


def kernel(left_graph_emb, right_graph_emb, left_x_batch, right_x_batch):
    global LAST_RESULT
    L = np.ascontiguousarray(np.asarray(left_graph_emb, dtype=np.float32))
    R = np.ascontiguousarray(np.asarray(right_graph_emb, dtype=np.float32))
    lb = np.asarray(left_x_batch).astype(np.int64)
    rb = np.asarray(right_x_batch).astype(np.int64)

    maxseg = max(int(np.bincount(lb, minlength=B).max()),
                 int(np.bincount(rb, minlength=B).max()))
    p = max(128, -(-maxseg // 128) * 128)  # pad width, multiple of 128
    Lp = _pack_side(L, lb, p)   # [B, p, D]
    Rp = _pack_side(R, rb, p)

    key = (PAIRS_PER_CORE, p)
    if key not in _NC_CACHE:
        _NC_CACHE[key] = _build_bass(*key)
    nc = _NC_CACHE[key]

    f8np = mybir.dt.np(mybir.dt.float8e4)
    in_maps = []
    for c in range(N_CORES):
        Lc = Lp[c * PAIRS_PER_CORE:(c + 1) * PAIRS_PER_CORE]  # [pairs, p, D]
        Rc = Rp[c * PAIRS_PER_CORE:(c + 1) * PAIRS_PER_CORE]
        nat = lambda X: np.ascontiguousarray(
            X.transpose(1, 0, 2).astype(np.float16))          # [p, pairs, D]
        tr8 = lambda X: np.ascontiguousarray(
            X.reshape(PAIRS_PER_CORE, p, DC, 128)
            .transpose(3, 2, 0, 1).astype(f8np))              # [128, DC, pairs, p]
        in_maps.append({"lt8": tr8(Lc), "rt8": tr8(Rc),
                        "l_nat": nat(Lc), "r_nat": nat(Rc)})

    LAST_IN_MAPS.append(in_maps)
    res = run_bass_kernel_spmd(nc, in_maps, list(range(N_CORES)))
    LAST_RESULT = res

    if os.environ.get("KERNEL_BENCH"):
        reps = int(os.environ.get("KERNEL_BENCH_REPS", "20"))
        LAST_TIMING["kernel_wall_s"] = _bench_exec(nc, in_maps, reps)
        LAST_TIMING["overhead_wall_s"] = _noop_baseline(reps)

    outs_l, outs_r = [], []
    for c in range(N_CORES):
        ot = res.results[c]["out_t"]                  # [128(dlow), DC, 2, pairs]
        ol = ot[:, :, 0, :].transpose(1, 0, 2).reshape(D, PAIRS_PER_CORE).T
        orr = ot[:, :, 1, :].transpose(1, 0, 2).reshape(D, PAIRS_PER_CORE).T
        outs_l.append(ol)
        outs_r.append(orr)
    out_l = np.concatenate(outs_l, axis=0)
    out_r = np.concatenate(outs_r, axis=0)
    return out_l.astype(np.float32), out_r.astype(np.float32)


# revision 17
# speedup vs baseline: 1.2431x; 1.2431x over previous
"""MatchFilter (graph-pair cross-attention + gated segment sum) on 8 trn2 cores.

Math per graph pair b (reference):
    S = L_b @ R_b^T                      [nl, nr]
    P_row = softmax(S, axis=1);  P_col = softmax(S, axis=0)
    wl_i = sigmoid(<L_i, (P_row @ R)_i>) = sigmoid( (sum_j E_ij S_ij) / (sum_j E_ij) )
    wr_j analogously from S^T.
    out_l[b] = sum_i wl_i L_i ;  out_r[b] = sum_j wr_j R_j

Device algorithm (identities that keep every reduction cheap):
  * <L_i, right_atten_i> = (sum_j E_ij S_ij) / (sum_j E_ij): only row/col
    stats of S are needed; the attention matmul is never materialized.
  * Everything is computed from the SINGLE orientation e = exp(S - c):
      s_R = colsum(e), t_R = colsum(e*S)  -> nearly-free N=1 PE matmuls
            against a ones vector (contraction along partitions);
      t_L = rowsum(e*S) -> free accum_out of the per-pair DVE product;
      s_L = rowsum(e)   -> per-pair copy-with-accum on the otherwise idle
            GPSIMD engine (reads e from SBUF; GPSIMD cannot touch PSUM).
  * sigmoid(z) = 0.5 + 0.5*tanh(z/2); the affine is folded into the output
    matmuls: out[b] = 0.5*(sum_i L_i + sum_i tanh_i L_i) via two accumulated
    N=1 matmuls per (pair, side, d-chunk), and the 0.5 into the final
    PSUM->SBUF copy.
  * Score matmuls run in fp8e4 DoubleRow: one matmul per pair does the full
    256-deep contraction.  Score quantization (~1 unit of S) is harmless:
    z sits 30+ sigma into sigmoid saturation for S ~ N(0,256), while the
    final weighted sum uses exact fp16 embeddings.

Sharding: 64 pairs -> 8 cores x 8 pairs, fully local (data parallel over
pairs).  Host pre-swizzles per-core inputs: d-major fp8 for scores, node-major
fp16 for the output matmuls.  Input DMAs are spread over the DMA-capable
queues (SP, Pool), score halves first so compute starts early.
"""

import os
import numpy as np
from contextlib import ExitStack

import concourse.bass as bass
import concourse.bacc as bacc
import concourse.tile as tile
from concourse import mybir
from concourse.bass_utils import run_bass_kernel_spmd

N_CORES = 8
B = 64            # graph pairs
D = 256           # embedding dim
DC = D // 128     # d-chunks for the 128-deep contraction
PAIRS_PER_CORE = B // N_CORES
# exp(S - EXP_SHIFT): keeps e^S and its row-sums inside f32 range for
# S ~ N(0, 256) (|S| <~ 95).  Max-subtraction is unnecessary because the
# z = t/s ratio is shift-invariant.
EXP_SHIFT = 32.0

LAST_RESULT = None  # BassKernelResults of the most recent run (for test.py)
LAST_TIMING = {}
LAST_IN_MAPS = []

_NC_CACHE = {}


def _build_bass(pairs: int, p: int):
    """Per-core program: `pairs` graph pairs, each padded to `p` nodes/side."""
    f16, f32 = mybir.dt.float16, mybir.dt.float32
    f8 = mybir.dt.float8e4
    AF = mybir.ActivationFunctionType
    ALU = mybir.AluOpType
    nl = p // 128  # row-chunks per pair side
    assert nl == 1, "kernel specialized for 128-node pairs"
    half = pairs // 2

    nc = bacc.Bacc("TRN2", target_bir_lowering=False, debug=False,
                   num_devices=N_CORES)
    lt8 = nc.dram_tensor("lt8", [128, DC, pairs, p], f8, kind="ExternalInput").ap()
    rt8 = nc.dram_tensor("rt8", [128, DC, pairs, p], f8, kind="ExternalInput").ap()
    l_nat = nc.dram_tensor("l_nat", [128, pairs, D], f16, kind="ExternalInput").ap()
    r_nat = nc.dram_tensor("r_nat", [128, pairs, D], f16, kind="ExternalInput").ap()
    out_t = nc.dram_tensor("out_t", [128, DC, 2, pairs], f32,
                           kind="ExternalOutput").ap()

    with tile.TileContext(nc) as tc, ExitStack() as ctx:
        sb = ctx.enter_context(tc.tile_pool(name="sb", bufs=1))
        work = ctx.enter_context(tc.tile_pool(name="work", bufs=1))
        stat = ctx.enter_context(tc.tile_pool(name="stat", bufs=1))
        psum = ctx.enter_context(tc.tile_pool(name="psum", bufs=1, space="PSUM"))
        spsum = ctx.enter_context(tc.tile_pool(name="spsum", bufs=1, space="PSUM"))

        lt_sb = sb.tile([128, DC, pairs, p], f8, tag="lt_sb")
        rt_sb = sb.tile([128, DC, pairs, p], f8, tag="rt_sb")
        ln_sb = sb.tile([128, pairs, D], f16, tag="ln_sb")
        rn_sb = sb.tile([128, pairs, D], f16, tag="rn_sb")

        neg_shift = stat.tile([128, 1], f32, tag="neg_shift")
        ones = stat.tile([128, 1], f32, tag="ones")
        ones16 = stat.tile([128, 1], f16, tag="ones16")
        nc.vector.memset(neg_shift, -EXP_SHIFT)
        nc.vector.memset(ones, 1.0)
        nc.vector.memset(ones16, 1.0)

        # input DMAs: score halves first on both queues so the first-half
        # scores are visible at the earliest possible time; nat tensors follow
        h0, h1 = slice(0, half), slice(half, pairs)
        nc.sync.dma_start(out=lt_sb[:, :, h0, :], in_=lt8[:, :, h0, :])
        nc.sync.dma_start(out=lt_sb[:, :, h1, :], in_=lt8[:, :, h1, :])
        nc.gpsimd.dma_start(out=rt_sb[:, :, h0, :], in_=rt8[:, :, h0, :])
        nc.gpsimd.dma_start(out=rt_sb[:, :, h1, :], in_=rt8[:, :, h1, :])
        nc.sync.dma_start(out=ln_sb, in_=l_nat)
        nc.gpsimd.dma_start(out=rn_sb, in_=r_nat)

        # dummy activation: pulls the exp_and_others ACT table load (Exp+Tanh)
        # off the critical path (overlaps the input DMA wait)
        warm = stat.tile([128, 1], f32, tag="warm")
        nc.scalar.activation(out=warm, in_=neg_shift, func=AF.Exp,
                             bias=neg_shift, scale=1.0)

        # right-side stats + s_L in one PSUM tile (PE colsum outputs);
        # t_L lands in SBUF via DVE accum_out
        stats = spsum.tile([128, 3, pairs], f32, tag="stats")
        stat_sl, stat_sr, stat_tr = (stats[:, 0, :], stats[:, 1, :],
                                     stats[:, 2, :])
        outT = spsum.tile([128, DC, 2, pairs], f32, tag="outT")
        stat_tl = stat.tile([128, pairs], f32, tag="stat_tl")
        ht = stat.tile([128, 2, pairs], f16, tag="ht")

        DR = mybir.MatmulPerfMode.DoubleRow
        st_all = psum.tile([128, pairs, p], f32, tag="st_all")
        et_all = work.tile([128, pairs, p], f32, tag="et_all")

        # e = exp(S - c) in [2, 2, 4]-pair chunks: the small early chunks
        # (entirely inside the first DMA half) start the DVE product chain as
        # early as possible.  Per-chunk tiles everywhere: shared tiles add
        # tile-granular RAW/WAR deps that stall chunk k on chunk k-1's work.
        chunks = [(0, 2), (2, 4), (4, 8)] if pairs == 8 else [(0, pairs)]
        for ci, (c0, c1) in enumerate(chunks):
            n = c1 - c0
            ss_c = psum.tile([128, n, p], f32, tag=f"ss_c{ci}")
            e_c = work.tile([128, n, p], f32, tag=f"e_c{ci}")
            sc_c = work.tile([128, n, p], f32, tag=f"sc_c{ci}")
            for b in range(c0, c1):
                nc.tensor.matmul(ss_c[:, b - c0, :], lhsT=lt_sb[:, :, b, :],
                                 rhs=rt_sb[:, :, b, :], perf_mode=DR,
                                 start=True, stop=True)
            nc.scalar.activation(out=e_c, in_=ss_c, func=AF.Exp,
                                 bias=neg_shift, scale=1.0)
            for b in range(c0, c1):
                i = b - c0
                # e*S product; free accum_out = t_L rowsums
                nc.vector.scalar_tensor_tensor(
                    out=sc_c[:, i, :], in0=e_c[:, i, :], scalar=1.0,
                    in1=ss_c[:, i, :], op0=ALU.mult, op1=ALU.mult,
                    accum_out=stat_tl[:, b:b + 1])
                # s_R / t_R colsums: nearly-free N=1 matmuls
                nc.tensor.matmul(stat_sr[:, b:b + 1], lhsT=e_c[:, i, :],
                                 rhs=ones, start=True, stop=True)
                nc.tensor.matmul(stat_tr[:, b:b + 1], lhsT=sc_c[:, i, :],
                                 rhs=ones, start=True, stop=True)

        # S^T orientation, only for s_L = colsum(exp(S^T)): one batched
        # matmul group + one batched exp + free colsums
        for b in range(pairs):
            nc.tensor.matmul(st_all[:, b, :], lhsT=rt_sb[:, :, b, :],
                             rhs=lt_sb[:, :, b, :], perf_mode=DR,
                             start=True, stop=True)
        nc.scalar.activation(out=et_all, in_=st_all, func=AF.Exp,
                             bias=neg_shift, scale=1.0)
        for b in range(pairs):
            nc.tensor.matmul(stat_sl[:, b:b + 1], lhsT=et_all[:, b, :],
                             rhs=ones, start=True, stop=True)

        # gates: ht = tanh((t/s)/2); sigmoid affine folded into the output
        # matmuls + final scaled copy
        z = stat.tile([128, 2, pairs], f32, tag="z")
        rcp = stat.tile([128, 2, pairs], f32, tag="rcp")
        nc.vector.reciprocal(out=rcp[:, 1, :], in_=stat_sr)
        nc.vector.tensor_mul(z[:, 1, :], stat_tr, rcp[:, 1, :])
        nc.vector.reciprocal(out=rcp[:, 0, :], in_=stat_sl)
        nc.vector.tensor_mul(z[:, 0, :], stat_tl, rcp[:, 0, :])
        nc.scalar.activation(out=ht, in_=z, func=AF.Tanh, scale=0.5)

        # gated segment sums: per (pair, side, d-chunk) an accumulated pair of
        # N=1 matmuls: psum = sum_i L_i + sum_i tanh_i * L_i  (= 2*out)
        for b in range(pairs):
            for c in range(DC):
                for side, nat in ((0, ln_sb), (1, rn_sb)):
                    col = outT[:, c, side, b:b + 1]
                    lhsT = nat[:, b, c * 128:(c + 1) * 128]
                    nc.tensor.matmul(col, lhsT=lhsT, rhs=ones16,
                                     start=True, stop=False)
                    nc.tensor.matmul(col, lhsT=lhsT,
                                     rhs=ht[:, side, b:b + 1],
                                     start=False, stop=True)

        out_sb = stat.tile([128, DC, 2, pairs], f32, tag="out_sb")
        nc.vector.tensor_scalar(out=out_sb, in0=outT, scalar1=0.5,
                                scalar2=None, op0=ALU.mult)
        nc.sync.dma_start(out=out_t, in_=out_sb)

    nc.compile()
    return nc


def _bench_exec(nc, in_maps, reps):
    """Min wall time of the cached jitted 8-core NEFF dispatch (inputs
    pre-sharded on device; excludes jax tracing and input upload)."""
    import time as _time
    import jax
    import jax.numpy as jnp
    from jax.sharding import Mesh, PartitionSpec, NamedSharding
    from jax.experimental.shard_map import shard_map
    from concourse import bass2jax
    from concourse.bass2jax import _bass_exec_p

    n_cores = len(in_maps)
    part_name = nc.partition_id_tensor.name if nc.partition_id_tensor else None
    in_names, out_names, out_avals = [], [], []
    for alloc in nc.m.functions[0].allocations:
        if not isinstance(alloc, mybir.MemoryLocationSet):
            continue
        name = alloc.memorylocations[0].name
        if alloc.kind == "ExternalInput":
            if name != part_name:
                in_names.append(name)
        elif alloc.kind == "ExternalOutput":
            out_names.append(name)
            out_avals.append(jax.core.ShapedArray(
                tuple(alloc.tensor_shape), mybir.dt.np(alloc.dtype)))
    n_params = len(in_names)
    all_in_names = in_names + out_names
    if part_name is not None:
        all_in_names = all_in_names + [part_name]

    def _body(*args):
        operands = list(args)
        if part_name is not None:
            operands.append(bass2jax.partition_id_tensor())
        return tuple(_bass_exec_p.bind(
            *operands, out_avals=tuple(out_avals), in_names=tuple(all_in_names),
            out_names=tuple(out_names), lowering_input_output_aliases=(),
            sim_require_finite=True, sim_require_nnan=True, nc=nc))

    devices = jax.devices()[:n_cores]
    mesh = Mesh(np.asarray(devices), ("core",))
    spec = PartitionSpec("core")
    fn = jax.jit(shard_map(_body, mesh=mesh,
                           in_specs=(spec,) * (n_params + len(out_names)),
                           out_specs=(spec,) * len(out_names)),
                 keep_unused=True)
    sharding = NamedSharding(mesh, spec)
    dev_ins = [jax.device_put(
        np.concatenate([np.asarray(m[name]) for m in in_maps], axis=0), sharding)
        for name in in_names]
    dev_zeros = [jax.device_put(
        np.zeros((n_cores * a.shape[0], *a.shape[1:]), a.dtype), sharding)
        for a in out_avals]
    fn(*dev_ins, *dev_zeros)[0].block_until_ready()  # warm compile
    best = float("inf")
    for _ in range(reps):
        t0 = _time.perf_counter()
        outs = fn(*dev_ins, *dev_zeros)
        for o in outs:
            o.block_until_ready()
        best = min(best, _time.perf_counter() - t0)
    return best


def _noop_baseline(reps):
    """Min wall time of a near-empty program through the same run path —
    estimates the host/axon dispatch overhead included in kernel_wall_s."""
    import time as _time
    if "noop" not in _NC_CACHE:
        nc = bacc.Bacc("TRN2", target_bir_lowering=False, debug=False,
                       num_devices=N_CORES)
        x = nc.dram_tensor("x", [128, 16], mybir.dt.float32,
                           kind="ExternalInput").ap()
        y = nc.dram_tensor("y", [128, 16], mybir.dt.float32,
                           kind="ExternalOutput").ap()
        with tile.TileContext(nc) as tc, ExitStack() as ctx:
            pool = ctx.enter_context(tc.tile_pool(name="p", bufs=1))
            t = pool.tile([128, 16], mybir.dt.float32)
            nc.sync.dma_start(out=t, in_=x)
            nc.sync.dma_start(out=y, in_=t)
        nc.compile()
        _NC_CACHE["noop"] = nc
    nc = _NC_CACHE["noop"]
    ins = [{"x": np.zeros((128, 16), np.float32)} for _ in range(N_CORES)]
    return _bench_exec(nc, ins, reps)


def sim_time_ns(in_map, pairs, p):
    """CoreSim cost-model time for one core's program (ns)."""
    from concourse import bass_interp
    key = (pairs, p)
    if key not in _NC_CACHE:
        _NC_CACHE[key] = _build_bass(*key)
    sim = bass_interp.CoreSim(_NC_CACHE[key])
    for name, arr in in_map.items():
        sim.tensor(name)[:] = arr
    sim.simulate()
    return int(sim.time)


def _pack_side(emb, seg_id, p):
    """[N, D] ragged -> [B, p, D] zero-padded f32 (no-copy reshape if uniform)."""
    counts = np.bincount(seg_id, minlength=B)
    if (counts == p).all():
        return emb.reshape(B, p, D)
    out = np.zeros((B, p, D), emb.dtype)
    offs = np.concatenate([[0], np.cumsum(counts)])
    for g in range(B):
        out[g, :counts[g]] = emb[offs[g]:offs[g + 1]]
    return out


def kernel(left_graph_emb, right_graph_emb, left_x_batch, right_x_batch):
    global LAST_RESULT
    L = np.ascontiguousarray(np.asarray(left_graph_emb, dtype=np.float32))
    R = np.ascontiguousarray(np.asarray(right_graph_emb, dtype=np.float32))
    lb = np.asarray(left_x_batch).astype(np.int64)
    rb = np.asarray(right_x_batch).astype(np.int64)

    maxseg = max(int(np.bincount(lb, minlength=B).max()),
                 int(np.bincount(rb, minlength=B).max()))
    p = max(128, -(-maxseg // 128) * 128)  # pad width, multiple of 128
    Lp = _pack_side(L, lb, p)   # [B, p, D]
    Rp = _pack_side(R, rb, p)

    key = (PAIRS_PER_CORE, p)
    if key not in _NC_CACHE:
        _NC_CACHE[key] = _build_bass(*key)
    nc = _NC_CACHE[key]

    f8np = mybir.dt.np(mybir.dt.float8e4)
    in_maps = []
    for c in range(N_CORES):
        Lc = Lp[c * PAIRS_PER_CORE:(c + 1) * PAIRS_PER_CORE]  # [pairs, p, D]
        Rc = Rp[c * PAIRS_PER_CORE:(c + 1) * PAIRS_PER_CORE]
        nat = lambda X: np.ascontiguousarray(
            X.transpose(1, 0, 2).astype(np.float16))          # [p, pairs, D]
        tr8 = lambda X: np.ascontiguousarray(
            X.reshape(PAIRS_PER_CORE, p, DC, 128)
            .transpose(3, 2, 0, 1).astype(f8np))              # [128, DC, pairs, p]
        in_maps.append({"lt8": tr8(Lc), "rt8": tr8(Rc),
                        "l_nat": nat(Lc), "r_nat": nat(Rc)})

    LAST_IN_MAPS.append(in_maps)
    res = run_bass_kernel_spmd(nc, in_maps, list(range(N_CORES)))
    LAST_RESULT = res

    if os.environ.get("KERNEL_BENCH"):
        reps = int(os.environ.get("KERNEL_BENCH_REPS", "20"))
        LAST_TIMING["kernel_wall_s"] = _bench_exec(nc, in_maps, reps)
        LAST_TIMING["overhead_wall_s"] = _noop_baseline(reps)

    outs_l, outs_r = [], []
    for c in range(N_CORES):
        ot = res.results[c]["out_t"]                  # [128(dlow), DC, 2, pairs]
        ol = ot[:, :, 0, :].transpose(1, 0, 2).reshape(D, PAIRS_PER_CORE).T
        orr = ot[:, :, 1, :].transpose(1, 0, 2).reshape(D, PAIRS_PER_CORE).T
        outs_l.append(ol)
        outs_r.append(orr)
    out_l = np.concatenate(outs_l, axis=0)
    out_r = np.concatenate(outs_r, axis=0)
    return out_l.astype(np.float32), out_r.astype(np.float32)
